# revision 25
# baseline (speedup 1.0000x reference)
"""Trainium2 Bass kernel for GQA attention (B=4, S=1024, D=4096, HQ=32, HKV=8).

Sharding: 8 cores = 4 batches x 2 head-groups. Each core computes one batch
with 16 q-heads / 4 kv-heads (Wq/Wk/Wv column-sharded, Wo row-sharded). The
two head-group partial outputs per batch are summed on the host, then
transposed (device emits out^T [Dout, S]) and bias bo added.

Device dataflow per core (all matmuls bf16):
  P1 (QKV) runs as 24 single-block column matmuls ordered [k,v,q0..q3] per
  kv group so attention for head h starts as soon as its q block is done.
  P2 (attention) is interleaved into P1: per q block, emit scores
  (kT-stationary, already-transposed scoresT), exp on ScalarE (unsafe
  softmax), causal mask on the diagonal, then the PREVIOUS head's
  av+den+normalize so the scalar-engine exp of head h overlaps the PE
  work of head h+1 and the remaining P1 blocks. av uses aT-stationary
  matmuls emitting o in [s1, d] layout; the softmax denominator reuses the
  same stationary with a ones column (1-cycle matmuls). Normalization is a
  single reciprocal + broadcast multiply per head; oT for P3 is produced by
  XBAR dma transposes (no PE time).
  P3 (Wo): out[m-chunk, s] accumulated over 16 head-chunks with
  Wo-row-stationary matmuls; Wo streamed once as 32 big DMAs.
"""

import math
import os

import numpy as np
import ml_dtypes

import concourse.bass as bass
import concourse.mybir as mybir
import concourse.tile as tile
from concourse import bacc
from concourse.bass_utils import run_bass_kernel_spmd
from concourse.masks import make_identity

B, S, D = 4, 1024, 4096
HQ, HKV, HD = 32, 8, 128
NH = 16          # q heads per core
NKV = 4          # kv heads per core
DQ = NH * HD     # 2048
DK = NKV * HD    # 512
NDC = D // 128   # 32 D-chunks
NSC = S // 128   # 8 s-chunks
QK_SCALE = 1.0 / math.sqrt(HD)

F32 = mybir.dt.float32
BF16 = mybir.dt.bfloat16

_GRAPH_CACHE = {}
LAST_PROFILE = None


def _block_specs():
    """24 P1 blocks in emission order: per kv group [k, v, q0..q3]."""
    blocks = []
    for g4 in range(NKV):
        blocks.append(("k", g4))
        blocks.append(("v", g4))
        for i in range(4):
            blocks.append(("q", 4 * g4 + i))
    return blocks


def _score_chunks(j):
    """Global (c0, c1) column chunks for k-chunk j (causal: s1 >= j*128),
    split at the 512 boundary so each chunk fits one psum bank."""
    if j < 4:
        return [(j * 128, 512), (512, 1024)]
    return [(j * 128, 1024)]


def _build_graph():
    nc = bacc.Bacc(debug=False)

    xt_ext = nc.dram_tensor("xt", [NDC, 128, S], BF16, kind="ExternalInput")
    # per-block weight columns, dc-major rows: [24, 128, NDC*128]
    wqkv_ext = nc.dram_tensor("wqkv", [24, 128, NDC * 128], BF16,
                              kind="ExternalInput")
    # Wo row-shard packed per output m-chunk: [32, 128, NH*128]
    wo_ext = nc.dram_tensor("wo", [NDC, 128, NH * 128], BF16,
                            kind="ExternalInput")
    cd1_ext = nc.dram_tensor("cd1", [128, S], BF16, kind="ExternalInput")
    cd2_ext = nc.dram_tensor("cd2", [128, S], BF16, kind="ExternalInput")
    # bias column tile: col bi = bias for block bi
    bqkv_ext = nc.dram_tensor("bqkv", [128, 24], F32, kind="ExternalInput")
    swp_ext = nc.dram_tensor("swp", [128, 128], BF16, kind="ExternalInput")
    out_ext = nc.dram_tensor("out", [D, S], F32, kind="ExternalOutput")
    debug = bool(os.environ.get("BASS_DEBUG_TAPS"))
    dbg_exts = {}
    if debug:
        dbg_exts["dbg_qT"] = nc.dram_tensor("dbg_qT", [128, NH * S], F32,
                                            kind="ExternalOutput")
        dbg_exts["dbg_kT"] = nc.dram_tensor("dbg_kT", [128, NKV * S], F32,
                                            kind="ExternalOutput")
        dbg_exts["dbg_v"] = nc.dram_tensor("dbg_v", [128, NSC * DK], F32,
                                           kind="ExternalOutput")
        dbg_exts["dbg_oT"] = nc.dram_tensor("dbg_oT", [128, NH * S], F32,
                                            kind="ExternalOutput")

    with tile.TileContext(nc) as tc:
        cpool = tc.alloc_tile_pool(name="const", bufs=1)
        ppool = tc.alloc_tile_pool(name="persist", bufs=1)
        wpool = tc.alloc_tile_pool(name="wk", bufs=1)
        xpool = tc.alloc_tile_pool(name="xts", bufs=1)
        psW = tc.alloc_tile_pool(name="psW", bufs=5, space="PSUM")
        psO = tc.alloc_tile_pool(name="psO", bufs=1, space="PSUM")

        # ---- constants ----
        maskT = cpool.tile([128, 128], BF16)   # 1 where s1 >= s2 else 0
        nc.gpsimd.memset(maskT[:], 1.0)
        nc.gpsimd.affine_select(
            out=maskT[:], in_=maskT[:], compare_op=mybir.AluOpType.is_ge,
            fill=0.0, base=0, pattern=[[1, 128]], channel_multiplier=-1)
        ident_b = cpool.tile([128, 128], BF16)
        make_identity(nc, ident_b)
        # -1024 where s2 > s1 (strictly upper): exp flushes these to 0
        maskbias = cpool.tile([128, 128], BF16)
        nc.gpsimd.memset(maskbias[:], 0.0)
        nc.gpsimd.affine_select(
            out=maskbias[:], in_=maskbias[:],
            compare_op=mybir.AluOpType.is_ge,
            fill=-1024.0, base=0, pattern=[[1, 128]], channel_multiplier=-1)
        ones_col = cpool.tile([128, 1], BF16)
        nc.gpsimd.memset(ones_col[:], 1.0)
        swp_sb = cpool.tile([128, 128], BF16)
        cd1_sb = cpool.tile([128, S], BF16)
        cd2_sb = cpool.tile([128, S], BF16)
        bias_sb = cpool.tile([128, 24], F32)

        # ---- persistent activations ----
        qT_all = ppool.tile([128, NH * S], BF16)    # [d, h*S + s]
        kT_all = ppool.tile([128, NKV * S], BF16)   # [d, g4*S + s]
        v_all = ppool.tile([128, NSC * DK], BF16)   # [s2-in-chunk, sc*DK + d]
        oT_all = ppool.tile([128, NH * S], BF16)    # [d, h*S + s]

        xts = [
            xpool.tile([128, S], BF16, tag=f"xt{dc}", name=f"xt{dc}")
            for dc in range(NDC)
        ]
        # block 0's weights must hit the DMA queues before the x flood;
        # split into 4 so the first dc chunks land quickly
        w0_t = wpool.tile([128, NDC * 128], BF16, tag="w", bufs=2, name="w_0")
        for p in range(4):
            nc.sync.dma_start(out=w0_t[:, p * 1024:(p + 1) * 1024],
                              in_=wqkv_ext[0, :, p * 1024:(p + 1) * 1024])
        engs = [nc.sync, nc.scalar, nc.gpsimd]
        for dc in range(NDC):
            engs[dc % 3].dma_start(out=xts[dc][:], in_=xt_ext[dc])
        nc.sync.dma_start(out=cd1_sb[:], in_=cd1_ext[:])
        nc.sync.dma_start(out=swp_sb[:], in_=swp_ext[:])
        nc.sync.dma_start(out=cd2_sb[:], in_=cd2_ext[:])
        nc.sync.dma_start(out=bias_sb[:], in_=bqkv_ext[:])

        def emit_p1_block(bi, kind, idx):
            """Emits the block matmuls and the half-0 evict/rope chain.
            Returns a closure emitting the half-1 rope tail (or the second
            half of v transposes) which the caller places where the PE has
            other work to hide the evict latency."""
            if bi == 0:
                w_t = w0_t
            else:
                w_t = wpool.tile([128, NDC * 128], BF16, tag="w", bufs=2,
                                 name=f"w{bi}")
                nc.sync.dma_start(out=w_t[:], in_=wqkv_ext[bi])
            acc0 = psW.tile([128, 512], F32, tag="acc", bufs=2,
                            name=f"acc0_{bi}")
            acc1 = psW.tile([128, 512], F32, tag="acc", bufs=2,
                            name=f"acc1_{bi}")
            tf = wpool.tile([128, S], BF16, tag="tf", bufs=2, name=f"tf{bi}")
            if kind != "v":
                dstT, col = (kT_all, idx) if kind == "k" else (qT_all, idx)
                t1 = wpool.tile([128, S], BF16, tag="rt0", bufs=2,
                                name=f"rt0_{bi}")

            def evict(hf, acc):
                nc.vector.tensor_scalar(
                    out=tf[:, hf * 512:(hf + 1) * 512], in0=acc[:],
                    scalar1=bias_sb[:, bi:bi + 1], scalar2=None,
                    op0=mybir.AluOpType.add)

            def rope(hf):
                # swap halves via PE permutation matmul (no DMA), then
                # dst = tf*cd1 + swapped*cd2 on DVE (t2 reads psum)
                cs = slice(hf * 512, (hf + 1) * 512)
                ps_swp = psW.tile([128, 512], F32, tag="sc", bufs=3,
                                  name=f"sw{bi}_{hf}")
                nc.tensor.matmul(ps_swp[:], swp_sb[:], tf[:, cs],
                                 start=True, stop=True)
                nc.vector.tensor_tensor(
                    out=t1[:, cs], in0=tf[:, cs], in1=cd1_sb[:, cs],
                    op=mybir.AluOpType.mult)
                t2 = wpool.tile([128, 512], BF16, tag="rt1", bufs=2,
                                name=f"rt1_{bi}_{hf}")
                nc.vector.tensor_tensor(
                    out=t2[:], in0=ps_swp[:], in1=cd2_sb[:, cs],
                    op=mybir.AluOpType.mult)
                nc.vector.tensor_tensor(
                    out=dstT[:, col * S + hf * 512:col * S + (hf + 1) * 512],
                    in0=t1[:, cs], in1=t2[:], op=mybir.AluOpType.add)

            def transposes(scs):
                for sc in scs:
                    tp = psW.tile([128, 128], BF16, tag="sc", bufs=3,
                                  name=f"tp{bi}_{sc}")
                    nc.tensor.transpose(
                        tp[:], tf[:, sc * 128:(sc + 1) * 128], ident_b)
                    nc.scalar.copy(
                        v_all[:, sc * DK + idx * 128:
                              sc * DK + (idx + 1) * 128],
                        tp[:])

            # half 0 accumulates fully first so its evict/rope chain runs
            # during half 1's matmuls
            for dc in range(NDC):
                nc.tensor.matmul(acc0[:], w_t[:, dc * 128:(dc + 1) * 128],
                                 xts[dc][:, 0:512],
                                 start=(dc == 0), stop=(dc == NDC - 1))
            evict(0, acc0)
            for dc in range(NDC):
                nc.tensor.matmul(acc1[:], w_t[:, dc * 128:(dc + 1) * 128],
                                 xts[dc][:, 512:1024],
                                 start=(dc == 0), stop=(dc == NDC - 1))
                if dc == 6 and kind != "v":
                    rope(0)  # tf half 0 ready: swap matmul hides here
                if dc == 20 and kind == "v":
                    transposes(range(4))
            evict(1, acc1)
            if kind == "v":
                return lambda: transposes(range(4, NSC))
            return lambda: rope(1)

        def emit_scores_exps(h):
            """scoresT_j = kT_j.T @ qT per 512-chunk (half-0 column chunks
            first: their qT rope half finishes earlier); exp -> aT; causal
            mask on the diagonal 128-block (gpsimd). Returns
            {(j, ci): (aT tile, c0)}."""
            g4 = h // 4
            chunks = ([(j, 0) for j in range(4)] + [(j, 1) for j in range(4)]
                      + [(j, 0) for j in range(4, NSC)])
            aT = {}
            for j, ci in chunks:
                c0, c1 = _score_chunks(j)[ci]
                w = c1 - c0
                scp = psW.tile([128, w], F32, tag="sc", bufs=3,
                               name=f"scp{h}_{j}_{ci}")
                nc.tensor.matmul(
                    scp[:], kT_all[:, g4 * S + j * 128:g4 * S + (j + 1) * 128],
                    qT_all[:, h * S + c0:h * S + c1],
                    start=True, stop=True)
                a = wpool.tile([128, w], BF16, tag="aT", bufs=12,
                               name=f"aT{h}_{j}_{ci}")
                nc.scalar.activation(
                    a[:], scp[:], mybir.ActivationFunctionType.Exp,
                    scale=QK_SCALE)
                if ci == 0:
                    # causal mask on diagonal block (local cols 0:128)
                    nc.gpsimd.tensor_tensor(
                        out=a[:, 0:128], in0=a[:, 0:128], in1=maskT[:],
                        op=mybir.AluOpType.mult)
                aT[(j, ci)] = (a, c0)
            return aT

        def emit_av(h, aT):
            """o[s1c, d] += aT_j[:, s1c].T @ v_j, den via same stationary with
            a ones column; then normalize + XBAR-transpose into oT_all."""
            g4 = h // 4
            o_ps = psO.tile([128, NSC, 128], F32, tag="o", bufs=1,
                            name=f"o_{h}")
            den_ps = psO.tile([128, NSC], F32, tag="den", bufs=1,
                              name=f"den_{h}")
            rcp = wpool.tile([128, NSC, 1], F32, tag="rcp", bufs=2,
                             name=f"rcp{h}")
            o_sb = wpool.tile([128, NSC, 128], BF16, tag="osb", bufs=2,
                              name=f"osb{h}")
            _, rb = bass.broadcast_tensor_aps(o_sb[:], rcp[:])
            # s1c-outer so each psum bank has one open accumulation group
            # at a time (av in o banks, den in its own bank, alternating);
            # normalize + transpose each 4-chunk half as soon as it is done
            for half in range(2):
                lo = half * 4
                for s1c in range(lo, lo + 4):
                    for j in range(s1c + 1):
                        vs = v_all[:, j * DK + g4 * 128:
                                   j * DK + (g4 + 1) * 128]
                        # find chunk holding global col s1c*128
                        if j < 4 and s1c >= 4:
                            a, c0 = aT[(j, 1)]
                        else:
                            a, c0 = aT[(j, 0)]
                        loc = s1c * 128 - c0
                        lhs = a[:, loc:loc + 128]
                        nc.tensor.matmul(
                            o_ps[:, s1c, :], lhs, vs,
                            start=(j == 0), stop=(j == s1c),
                            skip_group_check=True)
                        nc.tensor.matmul(
                            den_ps[:, s1c:s1c + 1], lhs, ones_col[:],
                            start=(j == 0), stop=(j == s1c),
                            skip_group_check=True)
                nc.vector.reciprocal(rcp[:, lo:lo + 4, 0],
                                     den_ps[:, lo:lo + 4])
                nc.vector.tensor_tensor(
                    out=o_sb[:, lo:lo + 4, :], in0=o_ps[:, lo:lo + 4, :],
                    in1=rb[:, lo:lo + 4, :], op=mybir.AluOpType.mult)
                for s1c in range(lo, lo + 4):
                    nc.sync.dma_start(
                        out=oT_all[:, h * S + s1c * 128:
                                   h * S + (s1c + 1) * 128],
                        in_=o_sb[:, s1c, :], transpose=True)

        def fetch_wo(m):
            wo_m = wpool.tile([128, NH, 128], BF16, tag="wo", bufs=3,
                              name=f"wo{m}")
            nc.gpsimd.dma_start(out=wo_m[:], in_=wo_ext[m])
            return wo_m

        # ---------------- P1 + P2 interleaved ----------------
        pending = None   # (head, aT dict) awaiting av emission
        deferred = None  # half-1 tail of the previous k/v block
        for bi, (kind, idx) in enumerate(_block_specs()):
            tail1 = emit_p1_block(bi, kind, idx)
            if deferred is not None:
                deferred()
                deferred = None
            if kind == "q":
                if pending is not None:
                    emit_av(*pending)
                tail1()  # q half-1 rope must precede its own scores
                pending = (idx, emit_scores_exps(idx))
            else:
                deferred = tail1
        wo_tiles = {m: fetch_wo(m) for m in range(2)}
        emit_av(*pending)

        if debug:
            for nm, t in [("dbg_qT", qT_all), ("dbg_kT", kT_all),
                          ("dbg_v", v_all), ("dbg_oT", oT_all)]:
                nc.gpsimd.dma_start(out=dbg_exts[nm][:], in_=t[:])

        psO.release()
        psW.release()
        xpool.release()
        cpool.seal()
        ppool.seal()

        # ---------------- P3: Wo ----------------
        p3sb = tc.alloc_tile_pool(name="p3sb", bufs=1)
        psP3 = tc.alloc_tile_pool(name="psP3", bufs=2, space="PSUM")
        for m in range(NDC):
            wo_m = wo_tiles.pop(m) if m in wo_tiles else fetch_wo(m)
            acc = psP3.tile([128, S], F32, tag="wps", bufs=2, name=f"wp{m}")
            for c in range(NH):
                lhs = wo_m[:, c, :]
                nc.tensor.matmul(acc[:, 0:512], lhs,
                                 oT_all[:, c * S:c * S + 512],
                                 start=(c == 0), stop=(c == NH - 1))
                nc.tensor.matmul(acc[:, 512:1024], lhs,
                                 oT_all[:, c * S + 512:c * S + 1024],
                                 start=(c == 0), stop=(c == NH - 1))
            oev = p3sb.tile([128, S], F32, tag="oev", bufs=2,
                            name=f"oev{m}")
            nc.scalar.copy(oev[:], acc[:])
            nc.sync.dma_start(
                out=out_ext[m * 128:(m + 1) * 128, :], in_=oev[:])
        wpool.seal()
        p3sb.release()
        psP3.release()

    nc.compile()
    return nc


def _ev(base):
    return np.concatenate([np.arange(base, base + HD, 2),
                           np.arange(base + 1, base + HD, 2)])


def _pack_wblock(Wcols):
    # [D, 128] -> [128, NDC*128] with dc-major columns
    return Wcols.reshape(NDC, 128, 128).transpose(1, 0, 2).reshape(128, -1)


def kernel(x, freqs_cis, Wq, bq, Wk, bk, Wv, bv, Wo, bo, startpos):
    global LAST_PROFILE
    x = np.asarray(x, dtype=np.float32)
    freqs_cis = np.asarray(freqs_cis, dtype=np.float32)
    Wq = np.asarray(Wq, dtype=np.float32)
    Wk = np.asarray(Wk, dtype=np.float32)
    Wv = np.asarray(Wv, dtype=np.float32)
    Wo = np.asarray(Wo, dtype=np.float32)
    bq = np.asarray(bq, dtype=np.float32)
    bk = np.asarray(bk, dtype=np.float32)
    bv = np.asarray(bv, dtype=np.float32)
    bo = np.asarray(bo, dtype=np.float32)
    assert int(startpos) == 0

    bf = lambda a: np.ascontiguousarray(a.astype(ml_dtypes.bfloat16))
    f32c = lambda a: np.ascontiguousarray(a.astype(np.float32))

    # rope coefficients in [d, s] layout: C64[i, s] = fc[s, i, 0]
    C64 = freqs_cis[:, :, 0].T          # [64, S]
    D64 = freqs_cis[:, :, 1].T
    cd1 = bf(np.vstack([C64, C64]))
    cd2 = bf(np.vstack([-D64, D64]))
    swp = np.zeros((128, 128), np.float32)
    swp[(np.arange(128) + 64) % 128, np.arange(128)] = 1.0
    swp = bf(swp)

    in_maps = []
    for core in range(8):
        b, g = core // 2, core % 2
        if core < 2:  # weight shards depend only on g; reuse for later cores
            wblocks, bcols = [], []
            for kind, idx in _block_specs():
                if kind == "k":
                    sel = _ev((g * NKV + idx) * HD)
                    wblocks.append(_pack_wblock(Wk[:, sel]))
                    bcols.append(bk[sel])
                elif kind == "v":
                    base = (g * NKV + idx) * HD
                    sel = np.arange(base, base + HD)
                    wblocks.append(_pack_wblock(Wv[:, sel]))
                    bcols.append(bv[sel])
                else:
                    sel = _ev((g * NH + idx) * HD)
                    wblocks.append(_pack_wblock(Wq[:, sel]))
                    bcols.append(bq[sel])
            wqkv_h = bf(np.stack(wblocks))                  # [24, 128, 4096]
            bqkv = f32c(np.stack(bcols, axis=1))            # [128, 24]
            Wos = Wo[g * DQ:(g + 1) * DQ, :]                # [2048, 4096]
            wo_h = bf(np.stack([
                Wos[:, m * 128:(m + 1) * 128]
                .reshape(NH, 128, 128).transpose(1, 0, 2).reshape(128, -1)
                for m in range(NDC)
            ]))                                             # [32, 128, 2048]
        else:
            prev = in_maps[core - 2]
            wqkv_h, wo_h, bqkv = prev["wqkv"], prev["wo"], prev["bqkv"]
        xt_h = bf(x[b].T.reshape(NDC, 128, S))
        in_maps.append({
            "xt": xt_h, "wqkv": wqkv_h, "wo": wo_h,
            "cd1": cd1, "cd2": cd2, "bqkv": bqkv, "swp": swp,
        })

    if "nc" not in _GRAPH_CACHE:
        _GRAPH_CACHE["nc"] = _build_graph()
    nc = _GRAPH_CACHE["nc"]

    res = run_bass_kernel_spmd(
        nc, in_maps, core_ids=list(range(8)),
        trace=bool(os.environ.get("BASS_TRACE")))
    LAST_PROFILE = res

    out = np.empty((B, S, D), dtype=np.float32)
    for b in range(B):
        t = res.results[2 * b]["out"] + res.results[2 * b + 1]["out"]
        out[b] = t.T + bo[None, :]
    return out


# revision 26
# speedup vs baseline: 1.0063x; 1.0063x over previous
"""Trainium2 Bass kernel for GQA attention (B=4, S=1024, D=4096, HQ=32, HKV=8).

Sharding: 8 cores = 4 batches x 2 head-groups. Each core computes one batch
with 16 q-heads / 4 kv-heads (Wq/Wk/Wv column-sharded, Wo row-sharded). The
two head-group partial outputs per batch are summed on the host, then
transposed (device emits out^T [Dout, S]) and bias bo added.

Device dataflow per core (all matmuls bf16):
  P1 (QKV) runs as 24 single-block column matmuls ordered [k,v,q0..q3] per
  kv group so attention for head h starts as soon as its q block is done.
  P2 (attention) is interleaved into P1: per q block, emit scores
  (kT-stationary, already-transposed scoresT), exp on ScalarE (unsafe
  softmax), causal mask on the diagonal, then the PREVIOUS head's
  av+den+normalize so the scalar-engine exp of head h overlaps the PE
  work of head h+1 and the remaining P1 blocks. av uses aT-stationary
  matmuls emitting o in [s1, d] layout; the softmax denominator reuses the
  same stationary with a ones column (1-cycle matmuls). Normalization is a
  single reciprocal + broadcast multiply per head; oT for P3 is produced by
  XBAR dma transposes (no PE time).
  P3 (Wo): out[m-chunk, s] accumulated over 16 head-chunks with
  Wo-row-stationary matmuls; Wo streamed once as 32 big DMAs.
"""

import math
import os

import numpy as np
import ml_dtypes

import concourse.bass as bass
import concourse.mybir as mybir
import concourse.tile as tile
from concourse import bacc
from concourse.bass_utils import run_bass_kernel_spmd
from concourse.masks import make_identity

B, S, D = 4, 1024, 4096
HQ, HKV, HD = 32, 8, 128
NH = 16          # q heads per core
NKV = 4          # kv heads per core
DQ = NH * HD     # 2048
DK = NKV * HD    # 512
NDC = D // 128   # 32 D-chunks
NSC = S // 128   # 8 s-chunks
QK_SCALE = 1.0 / math.sqrt(HD)

F32 = mybir.dt.float32
BF16 = mybir.dt.bfloat16

_GRAPH_CACHE = {}
LAST_PROFILE = None


def _block_specs():
    """24 P1 blocks in emission order: per kv group [k, v, q0..q3]."""
    blocks = []
    for g4 in range(NKV):
        blocks.append(("k", g4))
        blocks.append(("v", g4))
        for i in range(4):
            blocks.append(("q", 4 * g4 + i))
    return blocks


def _score_chunks(j):
    """Global (c0, c1) column chunks for k-chunk j (causal: s1 >= j*128),
    split at the 512 boundary so each chunk fits one psum bank."""
    if j < 4:
        return [(j * 128, 512), (512, 1024)]
    return [(j * 128, 1024)]


def _build_graph():
    nc = bacc.Bacc(debug=False)

    xt_ext = nc.dram_tensor("xt", [NDC, 128, S], BF16, kind="ExternalInput")
    # per-block weight columns, dc-major rows: [24, 128, NDC*128]
    wqkv_ext = nc.dram_tensor("wqkv", [24, 128, NDC * 128], BF16,
                              kind="ExternalInput")
    # Wo row-shard packed per output m-chunk: [32, 128, NH*128]
    wo_ext = nc.dram_tensor("wo", [NDC, 128, NH * 128], BF16,
                            kind="ExternalInput")
    cd1_ext = nc.dram_tensor("cd1", [128, S], BF16, kind="ExternalInput")
    cd2_ext = nc.dram_tensor("cd2", [128, S], BF16, kind="ExternalInput")
    # bias column tile: col bi = bias for block bi
    bqkv_ext = nc.dram_tensor("bqkv", [128, 24], F32, kind="ExternalInput")
    swp_ext = nc.dram_tensor("swp", [128, 128], BF16, kind="ExternalInput")
    out_ext = nc.dram_tensor("out", [D, S], F32, kind="ExternalOutput")
    debug = bool(os.environ.get("BASS_DEBUG_TAPS"))
    dbg_exts = {}
    if debug:
        dbg_exts["dbg_qT"] = nc.dram_tensor("dbg_qT", [128, NH * S], F32,
                                            kind="ExternalOutput")
        dbg_exts["dbg_kT"] = nc.dram_tensor("dbg_kT", [128, NKV * S], F32,
                                            kind="ExternalOutput")
        dbg_exts["dbg_v"] = nc.dram_tensor("dbg_v", [128, NSC * DK], F32,
                                           kind="ExternalOutput")
        dbg_exts["dbg_oT"] = nc.dram_tensor("dbg_oT", [128, NH * S], F32,
                                            kind="ExternalOutput")

    with tile.TileContext(nc) as tc:
        cpool = tc.alloc_tile_pool(name="const", bufs=1)
        ppool = tc.alloc_tile_pool(name="persist", bufs=1)
        wpool = tc.alloc_tile_pool(name="wk", bufs=1)
        xpool = tc.alloc_tile_pool(name="xts", bufs=1)
        psW = tc.alloc_tile_pool(name="psW", bufs=5, space="PSUM")
        psO = tc.alloc_tile_pool(name="psO", bufs=1, space="PSUM")

        # ---- constants ----
        maskT = cpool.tile([128, 128], BF16)   # 1 where s1 >= s2 else 0
        nc.gpsimd.memset(maskT[:], 1.0)
        nc.gpsimd.affine_select(
            out=maskT[:], in_=maskT[:], compare_op=mybir.AluOpType.is_ge,
            fill=0.0, base=0, pattern=[[1, 128]], channel_multiplier=-1)
        ident_b = cpool.tile([128, 128], BF16)
        make_identity(nc, ident_b)
        # -1024 where s2 > s1 (strictly upper): exp flushes these to 0
        maskbias = cpool.tile([128, 128], BF16)
        nc.gpsimd.memset(maskbias[:], 0.0)
        nc.gpsimd.affine_select(
            out=maskbias[:], in_=maskbias[:],
            compare_op=mybir.AluOpType.is_ge,
            fill=-1024.0, base=0, pattern=[[1, 128]], channel_multiplier=-1)
        ones_col = cpool.tile([128, 1], BF16)
        nc.gpsimd.memset(ones_col[:], 1.0)
        swp_sb = cpool.tile([128, 128], BF16)
        cd1_sb = cpool.tile([128, S], BF16)
        cd2_sb = cpool.tile([128, S], BF16)
        bias_sb = cpool.tile([128, 24], F32)

        # ---- persistent activations ----
        qT_all = ppool.tile([128, NH * S], BF16)    # [d, h*S + s]
        kT_all = ppool.tile([128, NKV * S], BF16)   # [d, g4*S + s]
        v_all = ppool.tile([128, NSC * DK], BF16)   # [s2-in-chunk, sc*DK + d]
        oT_all = ppool.tile([128, NH * S], BF16)    # [d, h*S + s]

        xts = [
            xpool.tile([128, S], BF16, tag=f"xt{dc}", name=f"xt{dc}")
            for dc in range(NDC)
        ]
        # block 0 weights and x tiles interleaved in consumption order so
        # no queue serializes a later-needed transfer before an earlier one
        w0_t = wpool.tile([128, NDC * 128], BF16, tag="w", bufs=2, name="w_0")
        engs = [nc.sync, nc.scalar, nc.gpsimd]
        for p in range(4):
            nc.sync.dma_start(out=w0_t[:, p * 1024:(p + 1) * 1024],
                              in_=wqkv_ext[0, :, p * 1024:(p + 1) * 1024])
            for k in range(8):
                dc = p * 8 + k
                engs[(dc + 1) % 3].dma_start(out=xts[dc][:], in_=xt_ext[dc])
        nc.sync.dma_start(out=cd1_sb[:], in_=cd1_ext[:])
        nc.sync.dma_start(out=swp_sb[:], in_=swp_ext[:])
        nc.sync.dma_start(out=cd2_sb[:], in_=cd2_ext[:])
        nc.sync.dma_start(out=bias_sb[:], in_=bqkv_ext[:])

        def emit_p1_block(bi, kind, idx):
            """Emits the block matmuls and the half-0 evict/rope chain.
            Returns a closure emitting the half-1 rope tail (or the second
            half of v transposes) which the caller places where the PE has
            other work to hide the evict latency."""
            if bi == 0:
                w_t = w0_t
            else:
                w_t = wpool.tile([128, NDC * 128], BF16, tag="w", bufs=2,
                                 name=f"w{bi}")
                nc.sync.dma_start(out=w_t[:], in_=wqkv_ext[bi])
            acc0 = psW.tile([128, 512], F32, tag="acc", bufs=2,
                            name=f"acc0_{bi}")
            acc1 = psW.tile([128, 512], F32, tag="acc", bufs=2,
                            name=f"acc1_{bi}")
            tf = wpool.tile([128, S], BF16, tag="tf", bufs=2, name=f"tf{bi}")
            if kind != "v":
                dstT, col = (kT_all, idx) if kind == "k" else (qT_all, idx)
                t1 = wpool.tile([128, S], BF16, tag="rt0", bufs=2,
                                name=f"rt0_{bi}")

            def evict(hf, acc):
                nc.vector.tensor_scalar(
                    out=tf[:, hf * 512:(hf + 1) * 512], in0=acc[:],
                    scalar1=bias_sb[:, bi:bi + 1], scalar2=None,
                    op0=mybir.AluOpType.add)

            def rope(hf):
                # swap halves via PE permutation matmul (no DMA), then
                # dst = tf*cd1 + swapped*cd2 on DVE (t2 reads psum)
                cs = slice(hf * 512, (hf + 1) * 512)
                ps_swp = psW.tile([128, 512], F32, tag="sc", bufs=3,
                                  name=f"sw{bi}_{hf}")
                nc.tensor.matmul(ps_swp[:], swp_sb[:], tf[:, cs],
                                 start=True, stop=True)
                nc.vector.tensor_tensor(
                    out=t1[:, cs], in0=tf[:, cs], in1=cd1_sb[:, cs],
                    op=mybir.AluOpType.mult)
                t2 = wpool.tile([128, 512], BF16, tag="rt1", bufs=2,
                                name=f"rt1_{bi}_{hf}")
                nc.vector.tensor_tensor(
                    out=t2[:], in0=ps_swp[:], in1=cd2_sb[:, cs],
                    op=mybir.AluOpType.mult)
                nc.vector.tensor_tensor(
                    out=dstT[:, col * S + hf * 512:col * S + (hf + 1) * 512],
                    in0=t1[:, cs], in1=t2[:], op=mybir.AluOpType.add)

            def transposes(scs):
                for sc in scs:
                    tp = psW.tile([128, 128], BF16, tag="sc", bufs=3,
                                  name=f"tp{bi}_{sc}")
                    nc.tensor.transpose(
                        tp[:], tf[:, sc * 128:(sc + 1) * 128], ident_b)
                    nc.scalar.copy(
                        v_all[:, sc * DK + idx * 128:
                              sc * DK + (idx + 1) * 128],
                        tp[:])

            # half 0 accumulates fully first so its evict/rope chain runs
            # during half 1's matmuls
            for dc in range(NDC):
                nc.tensor.matmul(acc0[:], w_t[:, dc * 128:(dc + 1) * 128],
                                 xts[dc][:, 0:512],
                                 start=(dc == 0), stop=(dc == NDC - 1))
            evict(0, acc0)
            for dc in range(NDC):
                nc.tensor.matmul(acc1[:], w_t[:, dc * 128:(dc + 1) * 128],
                                 xts[dc][:, 512:1024],
                                 start=(dc == 0), stop=(dc == NDC - 1))
                if dc == 6 and kind != "v":
                    rope(0)  # tf half 0 ready: swap matmul hides here
                if dc == 20 and kind == "v":
                    transposes(range(4))
            evict(1, acc1)
            if kind == "v":
                return lambda: transposes(range(4, NSC))
            return lambda: rope(1)

        def emit_scores_exps(h):
            """scoresT_j = kT_j.T @ qT per 512-chunk (half-0 column chunks
            first: their qT rope half finishes earlier); exp -> aT; causal
            mask on the diagonal 128-block (gpsimd). Returns
            {(j, ci): (aT tile, c0)}."""
            g4 = h // 4
            chunks = ([(j, 0) for j in range(4)] + [(j, 1) for j in range(4)]
                      + [(j, 0) for j in range(4, NSC)])
            aT = {}
            for j, ci in chunks:
                c0, c1 = _score_chunks(j)[ci]
                w = c1 - c0
                scp = psW.tile([128, w], F32, tag="sc", bufs=3,
                               name=f"scp{h}_{j}_{ci}")
                nc.tensor.matmul(
                    scp[:], kT_all[:, g4 * S + j * 128:g4 * S + (j + 1) * 128],
                    qT_all[:, h * S + c0:h * S + c1],
                    start=True, stop=True)
                a = wpool.tile([128, w], BF16, tag="aT", bufs=12,
                               name=f"aT{h}_{j}_{ci}")
                nc.scalar.activation(
                    a[:], scp[:], mybir.ActivationFunctionType.Exp,
                    scale=QK_SCALE)
                if ci == 0:
                    # causal mask on diagonal block (local cols 0:128)
                    nc.gpsimd.tensor_tensor(
                        out=a[:, 0:128], in0=a[:, 0:128], in1=maskT[:],
                        op=mybir.AluOpType.mult)
                aT[(j, ci)] = (a, c0)
            return aT

        def emit_av(h, aT):
            """o[s1c, d] += aT_j[:, s1c].T @ v_j, den via same stationary with
            a ones column; then normalize + XBAR-transpose into oT_all."""
            g4 = h // 4
            o_ps = psO.tile([128, NSC, 128], F32, tag="o", bufs=1,
                            name=f"o_{h}")
            den_ps = psO.tile([128, NSC], F32, tag="den", bufs=1,
                              name=f"den_{h}")
            rcp = wpool.tile([128, NSC, 1], F32, tag="rcp", bufs=2,
                             name=f"rcp{h}")
            o_sb = wpool.tile([128, NSC, 128], BF16, tag="osb", bufs=2,
                              name=f"osb{h}")
            _, rb = bass.broadcast_tensor_aps(o_sb[:], rcp[:])
            # s1c-outer so each psum bank has one open accumulation group
            # at a time (av in o banks, den in its own bank, alternating);
            # normalize + transpose each 4-chunk half as soon as it is done
            for half in range(2):
                lo = half * 4
                for s1c in range(lo, lo + 4):
                    for j in range(s1c + 1):
                        vs = v_all[:, j * DK + g4 * 128:
                                   j * DK + (g4 + 1) * 128]
                        # find chunk holding global col s1c*128
                        if j < 4 and s1c >= 4:
                            a, c0 = aT[(j, 1)]
                        else:
                            a, c0 = aT[(j, 0)]
                        loc = s1c * 128 - c0
                        lhs = a[:, loc:loc + 128]
                        nc.tensor.matmul(
                            o_ps[:, s1c, :], lhs, vs,
                            start=(j == 0), stop=(j == s1c),
                            skip_group_check=True)
                        nc.tensor.matmul(
                            den_ps[:, s1c:s1c + 1], lhs, ones_col[:],
                            start=(j == 0), stop=(j == s1c),
                            skip_group_check=True)
                nc.vector.reciprocal(rcp[:, lo:lo + 4, 0],
                                     den_ps[:, lo:lo + 4])
                nc.vector.tensor_tensor(
                    out=o_sb[:, lo:lo + 4, :], in0=o_ps[:, lo:lo + 4, :],
                    in1=rb[:, lo:lo + 4, :], op=mybir.AluOpType.mult)
                for s1c in range(lo, lo + 4):
                    nc.sync.dma_start(
                        out=oT_all[:, h * S + s1c * 128:
                                   h * S + (s1c + 1) * 128],
                        in_=o_sb[:, s1c, :], transpose=True)

        def fetch_wo(m):
            wo_m = wpool.tile([128, NH, 128], BF16, tag="wo", bufs=3,
                              name=f"wo{m}")
            nc.gpsimd.dma_start(out=wo_m[:], in_=wo_ext[m])
            return wo_m

        # ---------------- P1 + P2 interleaved ----------------
        pending = None   # (head, aT dict) awaiting av emission
        deferred = None  # half-1 tail of the previous k/v block
        for bi, (kind, idx) in enumerate(_block_specs()):
            tail1 = emit_p1_block(bi, kind, idx)
            if deferred is not None:
                deferred()
                deferred = None
            if kind == "q":
                if pending is not None:
                    emit_av(*pending)
                tail1()  # q half-1 rope must precede its own scores
                pending = (idx, emit_scores_exps(idx))
            else:
                deferred = tail1
        wo_tiles = {m: fetch_wo(m) for m in range(2)}
        emit_av(*pending)

        if debug:
            for nm, t in [("dbg_qT", qT_all), ("dbg_kT", kT_all),
                          ("dbg_v", v_all), ("dbg_oT", oT_all)]:
                nc.gpsimd.dma_start(out=dbg_exts[nm][:], in_=t[:])

        psO.release()
        psW.release()
        xpool.release()
        cpool.seal()
        ppool.seal()

        # ---------------- P3: Wo ----------------
        p3sb = tc.alloc_tile_pool(name="p3sb", bufs=1)
        psP3 = tc.alloc_tile_pool(name="psP3", bufs=2, space="PSUM")
        for m in range(NDC):
            wo_m = wo_tiles.pop(m) if m in wo_tiles else fetch_wo(m)
            acc = psP3.tile([128, S], F32, tag="wps", bufs=2, name=f"wp{m}")
            for c in range(NH):
                lhs = wo_m[:, c, :]
                nc.tensor.matmul(acc[:, 0:512], lhs,
                                 oT_all[:, c * S:c * S + 512],
                                 start=(c == 0), stop=(c == NH - 1))
                nc.tensor.matmul(acc[:, 512:1024], lhs,
                                 oT_all[:, c * S + 512:c * S + 1024],
                                 start=(c == 0), stop=(c == NH - 1))
            oev = p3sb.tile([128, S], F32, tag="oev", bufs=2,
                            name=f"oev{m}")
            nc.scalar.copy(oev[:], acc[:])
            nc.sync.dma_start(
                out=out_ext[m * 128:(m + 1) * 128, :], in_=oev[:])
        wpool.seal()
        p3sb.release()
        psP3.release()

    nc.compile()
    return nc


def _ev(base):
    return np.concatenate([np.arange(base, base + HD, 2),
                           np.arange(base + 1, base + HD, 2)])


def _pack_wblock(Wcols):
    # [D, 128] -> [128, NDC*128] with dc-major columns
    return Wcols.reshape(NDC, 128, 128).transpose(1, 0, 2).reshape(128, -1)


def kernel(x, freqs_cis, Wq, bq, Wk, bk, Wv, bv, Wo, bo, startpos):
    global LAST_PROFILE
    x = np.asarray(x, dtype=np.float32)
    freqs_cis = np.asarray(freqs_cis, dtype=np.float32)
    Wq = np.asarray(Wq, dtype=np.float32)
    Wk = np.asarray(Wk, dtype=np.float32)
    Wv = np.asarray(Wv, dtype=np.float32)
    Wo = np.asarray(Wo, dtype=np.float32)
    bq = np.asarray(bq, dtype=np.float32)
    bk = np.asarray(bk, dtype=np.float32)
    bv = np.asarray(bv, dtype=np.float32)
    bo = np.asarray(bo, dtype=np.float32)
    assert int(startpos) == 0

    bf = lambda a: np.ascontiguousarray(a.astype(ml_dtypes.bfloat16))
    f32c = lambda a: np.ascontiguousarray(a.astype(np.float32))

    # rope coefficients in [d, s] layout: C64[i, s] = fc[s, i, 0]
    C64 = freqs_cis[:, :, 0].T          # [64, S]
    D64 = freqs_cis[:, :, 1].T
    cd1 = bf(np.vstack([C64, C64]))
    cd2 = bf(np.vstack([-D64, D64]))
    swp = np.zeros((128, 128), np.float32)
    swp[(np.arange(128) + 64) % 128, np.arange(128)] = 1.0
    swp = bf(swp)

    in_maps = []
    for core in range(8):
        b, g = core // 2, core % 2
        if core < 2:  # weight shards depend only on g; reuse for later cores
            wblocks, bcols = [], []
            for kind, idx in _block_specs():
                if kind == "k":
                    sel = _ev((g * NKV + idx) * HD)
                    wblocks.append(_pack_wblock(Wk[:, sel]))
                    bcols.append(bk[sel])
                elif kind == "v":
                    base = (g * NKV + idx) * HD
                    sel = np.arange(base, base + HD)
                    wblocks.append(_pack_wblock(Wv[:, sel]))
                    bcols.append(bv[sel])
                else:
                    sel = _ev((g * NH + idx) * HD)
                    wblocks.append(_pack_wblock(Wq[:, sel]))
                    bcols.append(bq[sel])
            wqkv_h = bf(np.stack(wblocks))                  # [24, 128, 4096]
            bqkv = f32c(np.stack(bcols, axis=1))            # [128, 24]
            Wos = Wo[g * DQ:(g + 1) * DQ, :]                # [2048, 4096]
            wo_h = bf(np.stack([
                Wos[:, m * 128:(m + 1) * 128]
                .reshape(NH, 128, 128).transpose(1, 0, 2).reshape(128, -1)
                for m in range(NDC)
            ]))                                             # [32, 128, 2048]
        else:
            prev = in_maps[core - 2]
            wqkv_h, wo_h, bqkv = prev["wqkv"], prev["wo"], prev["bqkv"]
        xt_h = bf(x[b].T.reshape(NDC, 128, S))
        in_maps.append({
            "xt": xt_h, "wqkv": wqkv_h, "wo": wo_h,
            "cd1": cd1, "cd2": cd2, "bqkv": bqkv, "swp": swp,
        })

    if "nc" not in _GRAPH_CACHE:
        _GRAPH_CACHE["nc"] = _build_graph()
    nc = _GRAPH_CACHE["nc"]

    res = run_bass_kernel_spmd(
        nc, in_maps, core_ids=list(range(8)),
        trace=bool(os.environ.get("BASS_TRACE")))
    LAST_PROFILE = res

    out = np.empty((B, S, D), dtype=np.float32)
    for b in range(B):
        t = res.results[2 * b]["out"] + res.results[2 * b + 1]["out"]
        out[b] = t.T + bo[None, :]
    return out
